# revision 72
# baseline (speedup 1.0000x reference)
"""Trainium2 Bass kernel for nn_BlockB (dense_cnn: grouped 1-d convs + global max/mean pool).

Reference computation (B=128, D=300, L=128, F=8, window sizes 1/2/3):
  y_ws[b,d,f,l] = sum_k x[b,d,l+k] * W_ws[d*F+f,0,k] + b_ws[d*F+f]   (VALID conv)
  out[b, 0, ws, d, f] = max_l  y_ws        out[b, 1, ws, d, f] = mean_l y_ws

Strategy (pure data parallel over batch, 16 rows/core on 8 cores):
  * ws=2 and ws=3 max-pools: TensorE computes the conv outputs via block-diagonal
    "tap-stacked" matmuls (contraction over (d, tap) pairs; 16 channels x 8 filters
    = 128 output rows per group), DVE does segmented reduce_max straight out of PSUM.
    Bias is added after the max (max commutes with the constant bias).
  * ws=1 max-pool: max_l(W*x[l]) = max(W,0)*max_l(x) + min(W,0)*min_l(x), so only
    row max/min of x are needed.
  * mean pools are analytic: mean_l y = sum_k W_k * s_k / L' + bias, where the
    windowed sums s_k come from the full row sum S minus edge elements.
  * Host side only shards/reshapes inputs, packs weights into lhsT/coefficient
    tables, and reassembles the output - all O(weights) or pure layout work.
"""

import os
import sys

import numpy as np

for _p in ("/opt/trn_rl_repo", "/opt/pypackages"):
    if os.path.isdir(_p) and _p not in sys.path:
        sys.path.append(_p)

import concourse.bass as bass
import concourse.bacc as bacc
import concourse.bass_utils as _bass_utils
import concourse.mybir as mybir
from concourse.tile import TileContext
from concourse.tile_rust import add_dep_helper

# The repo's walrus invocation pins --enable-ldw-opt=false; this kernel issues
# 4 back-to-back matmuls per group with identical stationary weights, which
# ldw-opt dedupes (3 of 4 weight reloads elided). Rewrite the flag for our
# compiles only.
if not getattr(_bass_utils, "_ldw_opt_patched", False):
    _bass_utils._ldw_opt_patched = True
    _orig_run_command = _bass_utils.run_command

    def _run_command_ldw(argv, **kw):
        argv = [
            "--enable-ldw-opt=true" if a == "--enable-ldw-opt=false" else a
            for a in argv
        ]
        return _orig_run_command(argv, **kw)

    _bass_utils.run_command = _run_command_ldw

F32 = mybir.dt.float32
F32R = mybir.dt.float32r

# Problem constants (hardcoded per contest contract).
B, D, L, F = 128, 300, 128, 8
NCORES = 8
BLOC = B // NCORES          # 16 batch rows per core
DPAD = 304                  # pad D to 19*16
GD = 16                     # channels per matmul group
NG = DPAD // GD             # 19 groups
NROWS = BLOC * DPAD         # 4864 = 38 * 128 padded (b,d) rows
NT = NROWS // 128           # 38 row tiles
NQ = 4                      # PSUM bank chunks per group (4 b's each)
BQ = BLOC // NQ             # 4 b's per chunk

USE_F32R = True             # PE fast fp32 mode (1 cyc/row vs 4)
_MM_DT = F32R if USE_F32R else F32

AX = mybir.AxisListType
OP = mybir.AluOpType
ACTF = mybir.ActivationFunctionType


def _build_program():
    nc = bacc.Bacc("TRN2", target_bir_lowering=False, debug=False)

    # ---- DRAM parameters (per-core shard views; same NEFF on all cores) ----
    xrows_d = nc.declare_dram_parameter("xrows", [NROWS, L], F32, isOutput=False)
    # Host-interleaved rhs tiles: xw3s[g, k*GD+di, b, l] = x[b, g*GD+di, l+k]
    # (zero-padded beyond L). One dense contiguous DMA per group; the ws=2
    # matmuls reuse rows 0:32 (taps 0,1) of the same tile.
    xw3s_d = nc.declare_dram_parameter("xw3s", [NG, 3 * GD, BLOC, L], _MM_DT, isOutput=False)
    lhsT2_d = nc.declare_dram_parameter("lhsT2", [2 * GD, NG, 128], _MM_DT, isOutput=False)
    lhsT3_d = nc.declare_dram_parameter("lhsT3", [3 * GD, NG, 128], _MM_DT, isOutput=False)
    bias2_d = nc.declare_dram_parameter("bias2", [128, NG, BLOC], F32, isOutput=False)
    bias3_d = nc.declare_dram_parameter("bias3", [128, NG, BLOC], F32, isOutput=False)
    # Packed coefficient table for the per-row "smalls" pass; only each term's
    # live column range is stored. Layout along the last axis:
    #   0:32 Cb | 32:56 CS(j8:32) | 56:64 Cmx(j0:8) | 64:72 Cmn(j0:8)
    #   | 72:88 C0(j16:32) | 88:96 C1(j24:32) | 96:104 C126(j24:32) | 104:120 C127(j16:32)
    call_d = nc.declare_dram_parameter("call", [128, NT, 120], F32, isOutput=False)

    out2_d = nc.declare_dram_parameter("out2", [128, NG, BLOC], F32, isOutput=True)
    out3_d = nc.declare_dram_parameter("out3", [128, NG, BLOC], F32, isOutput=True)
    outs_d = nc.declare_dram_parameter("outs", [128, NT, 32], F32, isOutput=True)

    with TileContext(nc) as tc:
        with (
            tc.tile_pool(name="const", bufs=1) as cpool,
            tc.tile_pool(name="xw", bufs=3) as xwpool,
            tc.tile_pool(name="psum", bufs=2, space="PSUM") as pspool,
            tc.tile_pool(name="work", bufs=2) as wpool,
        ):
            # ---- persistent SBUF tensors ----
            # Constant/row loads go on the scalar/vector/tensor engine DMA
            # queues so the per-group xw streams (sync/gpsimd) start at t=0.
            # group-0 weight columns + first xrows half land first so both the
            # first matmuls and the first DVE row-reduces fire early.
            lhsT2 = cpool.tile([2 * GD, NG, 128], _MM_DT, tag="lhsT2", name="lhsT2")
            lhsT3 = cpool.tile([3 * GD, NG, 128], _MM_DT, tag="lhsT3", name="lhsT3")
            xrows = cpool.tile([128, NT, L], F32, tag="xrows", name="xrows")
            xrows_src = xrows_d.rearrange("(t p) l -> p t l", p=128)
            half = NT // 2
            nc.scalar.dma_start(out=lhsT2[:, 0:1], in_=lhsT2_d[:, 0:1])
            nc.scalar.dma_start(out=lhsT2[:, 1:NG], in_=lhsT2_d[:, 1:NG])
            nc.scalar.dma_start(out=lhsT3[:, 0:1], in_=lhsT3_d[:, 0:1])
            nc.scalar.dma_start(out=lhsT3[:, 1:NG], in_=lhsT3_d[:, 1:NG])
            xrh1 = nc.scalar.dma_start(out=xrows[:, 0:half, :], in_=xrows_src[:, 0:half, :])
            nc.scalar.dma_start(out=xrows[:, half:NT, :], in_=xrows_src[:, half:NT, :])

            call = cpool.tile([128, NT, 120], F32, tag="call", name="call")
            nc.scalar.dma_start(out=call[:, 0:half], in_=call_d[:, 0:half])
            nc.scalar.dma_start(out=call[:, half:NT], in_=call_d[:, half:NT])

            bias2 = cpool.tile([128, NG, BLOC], F32, tag="bias2", name="bias2")
            nc.scalar.dma_start(out=bias2[:], in_=bias2_d[:])
            bias3 = cpool.tile([128, NG, BLOC], F32, tag="bias3", name="bias3")
            nc.scalar.dma_start(out=bias3[:], in_=bias3_d[:])

            out2 = cpool.tile([128, NG, BLOC], F32, tag="out2", name="out2")
            out3 = cpool.tile([128, NG, BLOC], F32, tag="out3", name="out3")
            staging = cpool.tile([128, NT, 32], F32, tag="staging", name="staging")
            mxc = cpool.tile([128, NT, 1], F32, tag="mxc", name="mxc")
            mnc = cpool.tile([128, NT, 1], F32, tag="mnc", name="mnc")
            ssc = cpool.tile([128, NT, 1], F32, tag="ssc", name="ssc")

            # ---- ws=2 / ws=3 conv max-pools via PE + PSUM reduce ----
            # xw tiles carry 128 l-columns per (tap, b) so every DMA inner run is
            # a full 512 B (sub-512 B runs pay a 2x DMA latency penalty); the
            # matmuls read only the valid first L' columns. Tap k of the last
            # (padded) channels reads zeros from the host-padded xr.
            last_gp_dma = None
            g2_dma = None
            for g in range(NG):
                eng3 = nc.sync if g % 2 == 0 else nc.gpsimd
                xw3 = xwpool.tile([3 * GD, BLOC, 128], _MM_DT, tag="xw3", name="xw3")
                if g < 4:
                    # split the first tiles across both queues: faster pipeline start
                    hh = 3 * GD // 2
                    sd = nc.sync.dma_start(out=xw3[0:hh], in_=xw3s_d[g, 0:hh])
                    last_gp_dma = nc.gpsimd.dma_start(out=xw3[hh:], in_=xw3s_d[g, hh:])
                    if g == 2:
                        g2_dma = sd
                else:
                    di = eng3.dma_start(out=xw3[:], in_=xw3s_d[g])
                    if eng3 is nc.gpsimd:
                        last_gp_dma = di
                ps2 = pspool.tile([128, NQ, 512], F32, tag="ps", name="ps2")
                for q in range(NQ):
                    nc.tensor.matmul(
                        ps2[:, q, :],
                        lhsT2[:, g, :],
                        xw3[0 : 2 * GD, BQ * q : BQ * (q + 1), :],
                        start=True,
                        stop=True,
                    )
                # col l'=127 is the zero-pad tap position - excluded from the max
                nc.vector.tensor_reduce(
                    out2[:, g, :],
                    ps2[:].rearrange("p q (b l) -> p q b l", l=128)[:, :, :, 0:127],
                    axis=AX.X,
                    op=OP.max,
                )

                ps3 = pspool.tile([128, NQ, 512], F32, tag="ps", name="ps3")
                for q in range(NQ):
                    nc.tensor.matmul(
                        ps3[:, q, :],
                        lhsT3[:, g, :],
                        xw3[:, BQ * q : BQ * (q + 1), :],
                        start=True,
                        stop=True,
                    )
                nc.vector.tensor_reduce(
                    out3[:, g, :],
                    ps3[:].rearrange("p q (b l) -> p q b l", l=128)[:, :, :, 0:126],
                    axis=AX.X,
                    op=OP.max,
                )

            # Bias is added per result half (max commutes with bias); halves let
            # the first output stores overlap the second half of the pipeline.
            gh = 10
            nc.vector.tensor_tensor(out2[:, 0:gh], out2[:, 0:gh], bias2[:, 0:gh], op=OP.add)
            nc.vector.tensor_tensor(out3[:, 0:gh], out3[:, 0:gh], bias3[:, 0:gh], op=OP.add)
            nc.sync.dma_start(out=out2_d[:, 0:gh], in_=out2[:, 0:gh])
            nc.gpsimd.dma_start(out=out3_d[:, 0:gh], in_=out3[:, 0:gh])
            nc.vector.tensor_tensor(out2[:, gh:NG], out2[:, gh:NG], bias2[:, gh:NG], op=OP.add)
            nc.vector.tensor_tensor(out3[:, gh:NG], out3[:, gh:NG], bias3[:, gh:NG], op=OP.add)

            # ---- per-row reductions + smalls (ws1 max + all means) ----
            # 3-D reduces over the row tiles, split in halves so the first can
            # start as soon as the first xrows DMA lands.
            nc.vector.tensor_reduce(mxc[:, 0:half, :], xrows[:, 0:half, :], axis=AX.X, op=OP.max)
            nc.vector.tensor_reduce(mnc[:, 0:half, :], xrows[:, 0:half, :], axis=AX.X, op=OP.min)
            nc.vector.tensor_reduce(mxc[:, half:NT, :], xrows[:, half:NT, :], axis=AX.X, op=OP.max)
            nc.vector.tensor_reduce(mnc[:, half:NT, :], xrows[:, half:NT, :], axis=AX.X, op=OP.min)
            for t in range(NT):
                scr = wpool.tile([128, L], F32, tag="scr", name="scr")
                nc.scalar.activation(
                    scr[:], xrows[:, t, :], ACTF.Copy, accum_out=ssc[:, t, :]
                )

            # Smalls: staging[:, t, j] = sum_i call[:, t, i, j] * s_i[:, t] + Cb.
            # Broadcast the per-(row, tile) scalar along j with a stride-0 AP and
            # accumulate with tensor_tensor pairs, sliced to each term's live
            # column range ([0:8]=ws1max, [8:16]=mean1, [16:24]=mean2, [24:32]=mean3).
            tmp = cpool.tile([128, NT, 32], F32, tag="smtmp", name="smtmp")
            terms = [
                (ssc[:], 8, 32, 32),              # CS * S
                (mxc[:], 0, 8, 56),               # Cmx * max
                (mnc[:], 0, 8, 64),               # Cmn * min
                (xrows[:, :, 0:1], 16, 32, 72),   # C0 * x0
                (xrows[:, :, 1:2], 24, 32, 88),   # C1 * x1
                (xrows[:, :, 126:127], 24, 32, 96),   # C126 * x126
                (xrows[:, :, 127:128], 16, 32, 104),  # C127 * x127
            ]
            # Hold the bulk xrows transfer behind the first PE group tiles so
            # the PSUM pipeline's first ~400KB wins the HBM race at startup.
            if g2_dma is not None:
                add_dep_helper(xrh1.ins, g2_dma.ins, sync=True,
                               reason="xrows after first group tiles")

            sm_eng = nc.gpsimd  # keep the DVE free for the PSUM max-reduces
            sm_first = sm_eng.tensor_copy(staging[:], call[:, :, 0:32])  # Cb
            # Run the smalls chain only after the last gpsimd-issued xw3 DMA:
            # the chain otherwise blocks later groups' descriptor generation in
            # the gpsimd instruction stream and starves the PE.
            if last_gp_dma is not None:
                add_dep_helper(
                    sm_first.ins, last_gp_dma.ins, sync=False,
                    reason="smalls after group-stream descs",
                )
            for (s_ap, j0, j1, c0c) in terms:
                c_b, s_b = bass.broadcast_tensor_aps(
                    call[:, :, c0c : c0c + (j1 - j0)], s_ap
                )
                sm_eng.tensor_tensor(tmp[:, :, j0:j1], c_b, s_b, op=OP.mult)
                sm_eng.tensor_tensor(
                    staging[:, :, j0:j1], staging[:, :, j0:j1], tmp[:, :, j0:j1], op=OP.add
                )


            # ---- store (first halves of out2/out3 already stored above) ----
            nc.sync.dma_start(out=out2_d[:, gh:NG], in_=out2[:, gh:NG])
            nc.gpsimd.dma_start(out=out3_d[:, gh:NG], in_=out3[:, gh:NG])
            nc.sync.dma_start(out=outs_d[:], in_=staging[:])

    nc.compile()
    return nc


_PROGRAM_CACHE = {}


def _get_program():
    if "nc" not in _PROGRAM_CACHE:
        _PROGRAM_CACHE["nc"] = _build_program()
    return _PROGRAM_CACHE["nc"]


def _build_weight_tables(W1, b1, W2, b2, W3, b3):
    """All host-side packing of the (tiny) conv weights. Returns dict of arrays
    shared by every core."""
    W1v = W1[:, 0, :].reshape(D, F)            # [300, 8]
    W2v = W2[:, 0, :].reshape(D, F, 2)
    W3v = W3[:, 0, :].reshape(D, F, 3)
    b1v = b1.reshape(D, F)
    b2v = b2.reshape(D, F)
    b3v = b3.reshape(D, F)

    # lhsT for the tap-stacked block-diagonal matmuls.
    # lhsT[g, k*GD + di, di*F + f] = W[g*GD + di, f, k]
    def make_lhsT(Wv, ws):
        lhsT = np.zeros((NG, ws * GD, 128), np.float32)
        di = np.arange(GD)
        for g in range(NG):
            d = g * GD + di                      # [GD]
            valid = d < D
            dv = d[valid]
            div = di[valid]
            for k in range(ws):
                # rows k*GD+di ; cols di*F+f
                lhsT[g, k * GD + div[:, None], div[:, None] * F + np.arange(F)[None, :]] = Wv[
                    dv, :, k
                ]
        return lhsT

    # device wants [K, NG, 128] so the load is one dense DMA
    lhsT2 = np.ascontiguousarray(make_lhsT(W2v, 2).transpose(1, 0, 2))
    lhsT3 = np.ascontiguousarray(make_lhsT(W3v, 3).transpose(1, 0, 2))

    # bias tables for the PE outputs: row m = di*F+f, col (g, b); broadcast
    # along b so the device adds bias with a single tensor_tensor per ws.
    def make_bias(bv):
        out = np.zeros((128, NG), np.float32)
        for g in range(NG):
            d = g * GD + np.arange(GD)
            valid = d < D
            out[np.repeat(np.arange(GD)[valid], F) * F + np.tile(np.arange(F), valid.sum()), g] = bv[
                d[valid]
            ].reshape(-1)
        return np.repeat(out[:, :, None], BLOC, axis=2)  # [128, NG, BLOC]

    bias2 = make_bias(b2v)
    bias3 = make_bias(b3v)

    # Coefficient tables for the smalls pass. Row r = t*128 + p maps to
    # b = r // DPAD, d = r % DPAD. Columns: [0:8]=ws1 max, [8:16]=mean1,
    # [16:24]=mean2, [24:32]=mean3.
    r = np.arange(NROWS)
    d_of_r = r % DPAD                            # [NROWS]
    valid = d_of_r < D
    dsafe = np.where(valid, d_of_r, 0)

    def rowtab(vals):  # [NROWS, 8] -> [128, NT, 8]
        vals = np.where(valid[:, None], vals, 0.0)
        return vals.reshape(NT, 128, 8).transpose(1, 0, 2).astype(np.float32)

    W1r = W1v[dsafe]                             # [NROWS, 8]
    b1r = b1v[dsafe]
    W20, W21 = W2v[dsafe, :, 0], W2v[dsafe, :, 1]
    b2r = b2v[dsafe]
    W30, W31, W32 = W3v[dsafe, :, 0], W3v[dsafe, :, 1], W3v[dsafe, :, 2]
    b3r = b3v[dsafe]

    call = np.concatenate(
        [
            rowtab(b1r), rowtab(b1r), rowtab(b2r), rowtab(b3r),                # Cb   0:32
            rowtab(W1r / 128.0), rowtab((W20 + W21) / 127.0),
            rowtab((W30 + W31 + W32) / 126.0),                                 # CS  32:56
            rowtab(np.maximum(W1r, 0.0)),                                      # Cmx 56:64
            rowtab(np.minimum(W1r, 0.0)),                                      # Cmn 64:72
            rowtab(-W21 / 127.0), rowtab(-(W31 + W32) / 126.0),                # C0  72:88
            rowtab(-W32 / 126.0),                                              # C1  88:96
            rowtab(-W30 / 126.0),                                              # C126 96:104
            rowtab(-W20 / 127.0), rowtab(-(W30 + W31) / 126.0),                # C127 104:120
        ],
        axis=2,
    )  # [128, NT, 120]
    return {
        "lhsT2": lhsT2, "lhsT3": lhsT3, "bias2": bias2, "bias3": bias3, "call": call,
    }


def make_in_maps(input_data, W1, b1, W2, b2, W3, b3):
    """Build the 8 per-core input dicts."""
    tables = _build_weight_tables(
        np.asarray(W1, np.float32), np.asarray(b1, np.float32),
        np.asarray(W2, np.float32), np.asarray(b2, np.float32),
        np.asarray(W3, np.float32), np.asarray(b3, np.float32),
    )
    x = np.asarray(input_data, np.float32)
    in_maps = []
    for c in range(NCORES):
        xc = x[c * BLOC : (c + 1) * BLOC]               # [16, 300, 128]
        xp = np.zeros((BLOC, DPAD, L), np.float32)
        xp[:, :D] = xc
        xsh = np.zeros((3, BLOC, DPAD, L), np.float32)
        for k in range(3):
            xsh[k, :, :, : L - k] = xp[:, :, k:]
        # [k, b, d, l] -> per-group tap-stacked rhs tiles [g, k*GD+di, b, l]
        xg = xsh.reshape(3, BLOC, NG, GD, L)
        xw3s = np.ascontiguousarray(
            xg.transpose(2, 0, 3, 1, 4).reshape(NG, 3 * GD, BLOC, L)
        )
        m = {"xrows": xp.reshape(NROWS, L), "xw3s": xw3s}
        m.update(tables)
        in_maps.append(m)
    return in_maps


def assemble_output(results):
    """results: list of 8 dicts with out2/out3/outs -> full [B, 2, 3, D, F]."""
    out = np.empty((B, 2, 3, D, F), np.float32)
    for c, res in enumerate(results):
        bs = slice(c * BLOC, (c + 1) * BLOC)
        # outs: [128, NT, 32] ; row r = t*128+p -> (b = r//DPAD, d = r%DPAD)
        sm = np.ascontiguousarray(res["outs"].transpose(1, 0, 2)).reshape(BLOC, DPAD, 32)[:, :D]
        out[bs, 0, 0] = sm[:, :, 0:8]     # ws1 max
        out[bs, 1, 0] = sm[:, :, 8:16]    # ws1 mean
        out[bs, 1, 1] = sm[:, :, 16:24]   # ws2 mean
        out[bs, 1, 2] = sm[:, :, 24:32]   # ws3 mean
        # out2/out3: [128, NG, BLOC]; partition m = di*F+f, free = b
        for nm, wsi in (("out2", 1), ("out3", 2)):
            o = res[nm].reshape(GD, F, NG, BLOC).transpose(3, 2, 0, 1).reshape(
                BLOC, DPAD, F
            )[:, :D]
            out[bs, 0, wsi] = o
    return out


def kernel(input_data, W1, b1, W2, b2, W3, b3):
    from concourse.bass_utils import run_bass_kernel_spmd

    nc = _get_program()
    in_maps = make_in_maps(input_data, W1, b1, W2, b2, W3, b3)
    res = run_bass_kernel_spmd(nc, in_maps, list(range(NCORES)))
    return assemble_output(res.results)


# revision 74
# speedup vs baseline: 1.0265x; 1.0265x over previous
"""Trainium2 Bass kernel for nn_BlockB (dense_cnn: grouped 1-d convs + global max/mean pool).

Reference computation (B=128, D=300, L=128, F=8, window sizes 1/2/3):
  y_ws[b,d,f,l] = sum_k x[b,d,l+k] * W_ws[d*F+f,0,k] + b_ws[d*F+f]   (VALID conv)
  out[b, 0, ws, d, f] = max_l  y_ws        out[b, 1, ws, d, f] = mean_l y_ws

Strategy (pure data parallel over batch, 16 rows/core on 8 cores):
  * ws=2 and ws=3 max-pools: TensorE computes the conv outputs via block-diagonal
    "tap-stacked" matmuls (contraction over (d, tap) pairs; 16 channels x 8 filters
    = 128 output rows per group), DVE does segmented reduce_max straight out of PSUM.
    Bias is added after the max (max commutes with the constant bias).
  * ws=1 max-pool: max_l(W*x[l]) = max(W,0)*max_l(x) + min(W,0)*min_l(x), so only
    row max/min of x are needed.
  * mean pools are analytic: mean_l y = sum_k W_k * s_k / L' + bias, where the
    windowed sums s_k come from the full row sum S minus edge elements.
  * Host side only shards/reshapes inputs, packs weights into lhsT/coefficient
    tables, and reassembles the output - all O(weights) or pure layout work.
"""

import os
import sys

import numpy as np

for _p in ("/opt/trn_rl_repo", "/opt/pypackages"):
    if os.path.isdir(_p) and _p not in sys.path:
        sys.path.append(_p)

import concourse.bass as bass
import concourse.bacc as bacc
import concourse.bass_utils as _bass_utils
import concourse.mybir as mybir
from concourse.tile import TileContext
from concourse.tile_rust import add_dep_helper

# The repo's walrus invocation pins --enable-ldw-opt=false; this kernel issues
# 4 back-to-back matmuls per group with identical stationary weights, which
# ldw-opt dedupes (3 of 4 weight reloads elided). Rewrite the flag for our
# compiles only.
if not getattr(_bass_utils, "_ldw_opt_patched", False):
    _bass_utils._ldw_opt_patched = True
    _orig_run_command = _bass_utils.run_command

    def _run_command_ldw(argv, **kw):
        argv = [
            "--enable-ldw-opt=true" if a == "--enable-ldw-opt=false" else a
            for a in argv
        ]
        return _orig_run_command(argv, **kw)

    _bass_utils.run_command = _run_command_ldw

F32 = mybir.dt.float32
F32R = mybir.dt.float32r

# Problem constants (hardcoded per contest contract).
B, D, L, F = 128, 300, 128, 8
NCORES = 8
BLOC = B // NCORES          # 16 batch rows per core
DPAD = 304                  # pad D to 19*16
GD = 16                     # channels per matmul group
NG = DPAD // GD             # 19 groups
NROWS = BLOC * DPAD         # 4864 = 38 * 128 padded (b,d) rows
NT = NROWS // 128           # 38 row tiles
NQ = 4                      # PSUM bank chunks per group (4 b's each)
BQ = BLOC // NQ             # 4 b's per chunk

USE_F32R = True             # PE fast fp32 mode (1 cyc/row vs 4)
_MM_DT = F32R if USE_F32R else F32

AX = mybir.AxisListType
OP = mybir.AluOpType
ACTF = mybir.ActivationFunctionType


def _build_program():
    nc = bacc.Bacc("TRN2", target_bir_lowering=False, debug=False)

    # ---- DRAM parameters (per-core shard views; same NEFF on all cores) ----
    xrows_d = nc.declare_dram_parameter("xrows", [NROWS, L], F32, isOutput=False)
    # Host-interleaved rhs tiles: xw3s[g, k*GD+di, b, l] = x[b, g*GD+di, l+k]
    # (zero-padded beyond L). One dense contiguous DMA per group; the ws=2
    # matmuls reuse rows 0:32 (taps 0,1) of the same tile.
    xw3s_d = nc.declare_dram_parameter("xw3s", [NG, 3 * GD, BLOC, L], _MM_DT, isOutput=False)
    lhsT2_d = nc.declare_dram_parameter("lhsT2", [2 * GD, NG, 128], _MM_DT, isOutput=False)
    lhsT3_d = nc.declare_dram_parameter("lhsT3", [3 * GD, NG, 128], _MM_DT, isOutput=False)
    bias2_d = nc.declare_dram_parameter("bias2", [128, NG, BLOC], F32, isOutput=False)
    bias3_d = nc.declare_dram_parameter("bias3", [128, NG, BLOC], F32, isOutput=False)
    # Packed coefficient table for the per-row "smalls" pass; only each term's
    # live column range is stored. Layout along the last axis:
    #   0:32 Cb | 32:56 CS(j8:32) | 56:64 Cmx(j0:8) | 64:72 Cmn(j0:8)
    #   | 72:88 C0(j16:32) | 88:96 C1(j24:32) | 96:104 C126(j24:32) | 104:120 C127(j16:32)
    call_d = nc.declare_dram_parameter("call", [128, NT, 120], F32, isOutput=False)

    out2_d = nc.declare_dram_parameter("out2", [128, NG, BLOC], F32, isOutput=True)
    out3_d = nc.declare_dram_parameter("out3", [128, NG, BLOC], F32, isOutput=True)
    outs_d = nc.declare_dram_parameter("outs", [128, NT, 32], F32, isOutput=True)

    with TileContext(nc) as tc:
        with (
            tc.tile_pool(name="const", bufs=1) as cpool,
            tc.tile_pool(name="xw", bufs=3) as xwpool,
            tc.tile_pool(name="psum", bufs=2, space="PSUM") as pspool,
            tc.tile_pool(name="work", bufs=2) as wpool,
        ):
            # ---- persistent SBUF tensors ----
            # Constant/row loads go on the scalar/vector/tensor engine DMA
            # queues so the per-group xw streams (sync/gpsimd) start at t=0.
            # group-0 weight columns + first xrows half land first so both the
            # first matmuls and the first DVE row-reduces fire early.
            lhsT2 = cpool.tile([2 * GD, NG, 128], _MM_DT, tag="lhsT2", name="lhsT2")
            lhsT3 = cpool.tile([3 * GD, NG, 128], _MM_DT, tag="lhsT3", name="lhsT3")
            xrows = cpool.tile([128, NT, L], F32, tag="xrows", name="xrows")
            xrows_src = xrows_d.rearrange("(t p) l -> p t l", p=128)
            half = NT // 2
            nc.scalar.dma_start(out=lhsT2[:, 0:1], in_=lhsT2_d[:, 0:1])
            nc.scalar.dma_start(out=lhsT2[:, 1:NG], in_=lhsT2_d[:, 1:NG])
            nc.scalar.dma_start(out=lhsT3[:, 0:1], in_=lhsT3_d[:, 0:1])
            nc.scalar.dma_start(out=lhsT3[:, 1:NG], in_=lhsT3_d[:, 1:NG])
            nc.scalar.dma_start(out=xrows[:, 0:half, :], in_=xrows_src[:, 0:half, :])
            nc.scalar.dma_start(out=xrows[:, half:NT, :], in_=xrows_src[:, half:NT, :])

            call = cpool.tile([128, NT, 120], F32, tag="call", name="call")
            nc.scalar.dma_start(out=call[:, 0:half], in_=call_d[:, 0:half])
            nc.scalar.dma_start(out=call[:, half:NT], in_=call_d[:, half:NT])

            bias2 = cpool.tile([128, NG, BLOC], F32, tag="bias2", name="bias2")
            nc.scalar.dma_start(out=bias2[:], in_=bias2_d[:])
            bias3 = cpool.tile([128, NG, BLOC], F32, tag="bias3", name="bias3")
            nc.scalar.dma_start(out=bias3[:], in_=bias3_d[:])

            out2 = cpool.tile([128, NG, BLOC], F32, tag="out2", name="out2")
            out3 = cpool.tile([128, NG, BLOC], F32, tag="out3", name="out3")
            staging = cpool.tile([128, NT, 32], F32, tag="staging", name="staging")
            mxc = cpool.tile([128, NT, 1], F32, tag="mxc", name="mxc")
            mnc = cpool.tile([128, NT, 1], F32, tag="mnc", name="mnc")
            ssc = cpool.tile([128, NT, 1], F32, tag="ssc", name="ssc")

            # ---- ws=2 / ws=3 conv max-pools via PE + PSUM reduce ----
            # xw tiles carry 128 l-columns per (tap, b) so every DMA inner run is
            # a full 512 B (sub-512 B runs pay a 2x DMA latency penalty); the
            # matmuls read only the valid first L' columns. Tap k of the last
            # (padded) channels reads zeros from the host-padded xr.
            last_gp_dma = None
            for g in range(NG):
                eng3 = nc.sync if g % 2 == 0 else nc.gpsimd
                xw3 = xwpool.tile([3 * GD, BLOC, 128], _MM_DT, tag="xw3", name="xw3")
                if g == 0:
                    # quad-0's b-rows land first (~100KB) so the very first
                    # matmul+reduce fire as early as possible
                    nc.sync.dma_start(out=xw3[:, 0:BQ, :], in_=xw3s_d[g, :, 0:BQ, :])
                    nc.sync.dma_start(out=xw3[:, BQ : 2 * BQ, :], in_=xw3s_d[g, :, BQ : 2 * BQ, :])
                    last_gp_dma = nc.gpsimd.dma_start(
                        out=xw3[:, 2 * BQ :, :], in_=xw3s_d[g, :, 2 * BQ :, :]
                    )
                elif g < 4:
                    # split the first tiles across both queues: faster pipeline start
                    hh = 3 * GD // 2
                    nc.sync.dma_start(out=xw3[0:hh], in_=xw3s_d[g, 0:hh])
                    last_gp_dma = nc.gpsimd.dma_start(out=xw3[hh:], in_=xw3s_d[g, hh:])
                else:
                    di = eng3.dma_start(out=xw3[:], in_=xw3s_d[g])
                    if eng3 is nc.gpsimd:
                        last_gp_dma = di
                ps2 = pspool.tile([128, NQ, 512], F32, tag="ps", name="ps2")
                for q in range(NQ):
                    nc.tensor.matmul(
                        ps2[:, q, :],
                        lhsT2[:, g, :],
                        xw3[0 : 2 * GD, BQ * q : BQ * (q + 1), :],
                        start=True,
                        stop=True,
                    )
                # col l'=127 is the zero-pad tap position - excluded from the max
                nc.vector.tensor_reduce(
                    out2[:, g, :],
                    ps2[:].rearrange("p q (b l) -> p q b l", l=128)[:, :, :, 0:127],
                    axis=AX.X,
                    op=OP.max,
                )

                ps3 = pspool.tile([128, NQ, 512], F32, tag="ps", name="ps3")
                for q in range(NQ):
                    nc.tensor.matmul(
                        ps3[:, q, :],
                        lhsT3[:, g, :],
                        xw3[:, BQ * q : BQ * (q + 1), :],
                        start=True,
                        stop=True,
                    )
                nc.vector.tensor_reduce(
                    out3[:, g, :],
                    ps3[:].rearrange("p q (b l) -> p q b l", l=128)[:, :, :, 0:126],
                    axis=AX.X,
                    op=OP.max,
                )

            # Bias is added per result half (max commutes with bias); halves let
            # the first output stores overlap the second half of the pipeline.
            gh = 10
            nc.vector.tensor_tensor(out2[:, 0:gh], out2[:, 0:gh], bias2[:, 0:gh], op=OP.add)
            nc.vector.tensor_tensor(out3[:, 0:gh], out3[:, 0:gh], bias3[:, 0:gh], op=OP.add)
            nc.sync.dma_start(out=out2_d[:, 0:gh], in_=out2[:, 0:gh])
            nc.gpsimd.dma_start(out=out3_d[:, 0:gh], in_=out3[:, 0:gh])
            nc.vector.tensor_tensor(out2[:, gh:NG], out2[:, gh:NG], bias2[:, gh:NG], op=OP.add)
            nc.vector.tensor_tensor(out3[:, gh:NG], out3[:, gh:NG], bias3[:, gh:NG], op=OP.add)

            # ---- per-row reductions + smalls (ws1 max + all means) ----
            # 3-D reduces over the row tiles, split in halves so the first can
            # start as soon as the first xrows DMA lands.
            nc.vector.tensor_reduce(mxc[:, 0:half, :], xrows[:, 0:half, :], axis=AX.X, op=OP.max)
            nc.vector.tensor_reduce(mnc[:, 0:half, :], xrows[:, 0:half, :], axis=AX.X, op=OP.min)
            nc.vector.tensor_reduce(mxc[:, half:NT, :], xrows[:, half:NT, :], axis=AX.X, op=OP.max)
            nc.vector.tensor_reduce(mnc[:, half:NT, :], xrows[:, half:NT, :], axis=AX.X, op=OP.min)
            for t in range(NT):
                scr = wpool.tile([128, L], F32, tag="scr", name="scr")
                nc.scalar.activation(
                    scr[:], xrows[:, t, :], ACTF.Copy, accum_out=ssc[:, t, :]
                )

            # Smalls: staging[:, t, j] = sum_i call[:, t, i, j] * s_i[:, t] + Cb.
            # Broadcast the per-(row, tile) scalar along j with a stride-0 AP and
            # accumulate with tensor_tensor pairs, sliced to each term's live
            # column range ([0:8]=ws1max, [8:16]=mean1, [16:24]=mean2, [24:32]=mean3).
            tmp = cpool.tile([128, NT, 32], F32, tag="smtmp", name="smtmp")
            terms = [
                (ssc[:], 8, 32, 32),              # CS * S
                (mxc[:], 0, 8, 56),               # Cmx * max
                (mnc[:], 0, 8, 64),               # Cmn * min
                (xrows[:, :, 0:1], 16, 32, 72),   # C0 * x0
                (xrows[:, :, 1:2], 24, 32, 88),   # C1 * x1
                (xrows[:, :, 126:127], 24, 32, 96),   # C126 * x126
                (xrows[:, :, 127:128], 16, 32, 104),  # C127 * x127
            ]
            sm_eng = nc.gpsimd  # keep the DVE free for the PSUM max-reduces
            sm_first = sm_eng.tensor_copy(staging[:], call[:, :, 0:32])  # Cb
            # Run the smalls chain only after the last gpsimd-issued xw3 DMA:
            # the chain otherwise blocks later groups' descriptor generation in
            # the gpsimd instruction stream and starves the PE.
            if last_gp_dma is not None:
                add_dep_helper(
                    sm_first.ins, last_gp_dma.ins, sync=False,
                    reason="smalls after group-stream descs",
                )
            for (s_ap, j0, j1, c0c) in terms:
                c_b, s_b = bass.broadcast_tensor_aps(
                    call[:, :, c0c : c0c + (j1 - j0)], s_ap
                )
                sm_eng.tensor_tensor(tmp[:, :, j0:j1], c_b, s_b, op=OP.mult)
                sm_eng.tensor_tensor(
                    staging[:, :, j0:j1], staging[:, :, j0:j1], tmp[:, :, j0:j1], op=OP.add
                )


            # ---- store (first halves of out2/out3 already stored above) ----
            nc.sync.dma_start(out=out2_d[:, gh:NG], in_=out2[:, gh:NG])
            nc.gpsimd.dma_start(out=out3_d[:, gh:NG], in_=out3[:, gh:NG])
            nc.sync.dma_start(out=outs_d[:], in_=staging[:])

    nc.compile()
    return nc


_PROGRAM_CACHE = {}


def _get_program():
    if "nc" not in _PROGRAM_CACHE:
        _PROGRAM_CACHE["nc"] = _build_program()
    return _PROGRAM_CACHE["nc"]


def _build_weight_tables(W1, b1, W2, b2, W3, b3):
    """All host-side packing of the (tiny) conv weights. Returns dict of arrays
    shared by every core."""
    W1v = W1[:, 0, :].reshape(D, F)            # [300, 8]
    W2v = W2[:, 0, :].reshape(D, F, 2)
    W3v = W3[:, 0, :].reshape(D, F, 3)
    b1v = b1.reshape(D, F)
    b2v = b2.reshape(D, F)
    b3v = b3.reshape(D, F)

    # lhsT for the tap-stacked block-diagonal matmuls.
    # lhsT[g, k*GD + di, di*F + f] = W[g*GD + di, f, k]
    def make_lhsT(Wv, ws):
        lhsT = np.zeros((NG, ws * GD, 128), np.float32)
        di = np.arange(GD)
        for g in range(NG):
            d = g * GD + di                      # [GD]
            valid = d < D
            dv = d[valid]
            div = di[valid]
            for k in range(ws):
                # rows k*GD+di ; cols di*F+f
                lhsT[g, k * GD + div[:, None], div[:, None] * F + np.arange(F)[None, :]] = Wv[
                    dv, :, k
                ]
        return lhsT

    # device wants [K, NG, 128] so the load is one dense DMA
    lhsT2 = np.ascontiguousarray(make_lhsT(W2v, 2).transpose(1, 0, 2))
    lhsT3 = np.ascontiguousarray(make_lhsT(W3v, 3).transpose(1, 0, 2))

    # bias tables for the PE outputs: row m = di*F+f, col (g, b); broadcast
    # along b so the device adds bias with a single tensor_tensor per ws.
    def make_bias(bv):
        out = np.zeros((128, NG), np.float32)
        for g in range(NG):
            d = g * GD + np.arange(GD)
            valid = d < D
            out[np.repeat(np.arange(GD)[valid], F) * F + np.tile(np.arange(F), valid.sum()), g] = bv[
                d[valid]
            ].reshape(-1)
        return np.repeat(out[:, :, None], BLOC, axis=2)  # [128, NG, BLOC]

    bias2 = make_bias(b2v)
    bias3 = make_bias(b3v)

    # Coefficient tables for the smalls pass. Row r = t*128 + p maps to
    # b = r // DPAD, d = r % DPAD. Columns: [0:8]=ws1 max, [8:16]=mean1,
    # [16:24]=mean2, [24:32]=mean3.
    r = np.arange(NROWS)
    d_of_r = r % DPAD                            # [NROWS]
    valid = d_of_r < D
    dsafe = np.where(valid, d_of_r, 0)

    def rowtab(vals):  # [NROWS, 8] -> [128, NT, 8]
        vals = np.where(valid[:, None], vals, 0.0)
        return vals.reshape(NT, 128, 8).transpose(1, 0, 2).astype(np.float32)

    W1r = W1v[dsafe]                             # [NROWS, 8]
    b1r = b1v[dsafe]
    W20, W21 = W2v[dsafe, :, 0], W2v[dsafe, :, 1]
    b2r = b2v[dsafe]
    W30, W31, W32 = W3v[dsafe, :, 0], W3v[dsafe, :, 1], W3v[dsafe, :, 2]
    b3r = b3v[dsafe]

    call = np.concatenate(
        [
            rowtab(b1r), rowtab(b1r), rowtab(b2r), rowtab(b3r),                # Cb   0:32
            rowtab(W1r / 128.0), rowtab((W20 + W21) / 127.0),
            rowtab((W30 + W31 + W32) / 126.0),                                 # CS  32:56
            rowtab(np.maximum(W1r, 0.0)),                                      # Cmx 56:64
            rowtab(np.minimum(W1r, 0.0)),                                      # Cmn 64:72
            rowtab(-W21 / 127.0), rowtab(-(W31 + W32) / 126.0),                # C0  72:88
            rowtab(-W32 / 126.0),                                              # C1  88:96
            rowtab(-W30 / 126.0),                                              # C126 96:104
            rowtab(-W20 / 127.0), rowtab(-(W30 + W31) / 126.0),                # C127 104:120
        ],
        axis=2,
    )  # [128, NT, 120]
    return {
        "lhsT2": lhsT2, "lhsT3": lhsT3, "bias2": bias2, "bias3": bias3, "call": call,
    }


def make_in_maps(input_data, W1, b1, W2, b2, W3, b3):
    """Build the 8 per-core input dicts."""
    tables = _build_weight_tables(
        np.asarray(W1, np.float32), np.asarray(b1, np.float32),
        np.asarray(W2, np.float32), np.asarray(b2, np.float32),
        np.asarray(W3, np.float32), np.asarray(b3, np.float32),
    )
    x = np.asarray(input_data, np.float32)
    in_maps = []
    for c in range(NCORES):
        xc = x[c * BLOC : (c + 1) * BLOC]               # [16, 300, 128]
        xp = np.zeros((BLOC, DPAD, L), np.float32)
        xp[:, :D] = xc
        xsh = np.zeros((3, BLOC, DPAD, L), np.float32)
        for k in range(3):
            xsh[k, :, :, : L - k] = xp[:, :, k:]
        # [k, b, d, l] -> per-group tap-stacked rhs tiles [g, k*GD+di, b, l]
        xg = xsh.reshape(3, BLOC, NG, GD, L)
        xw3s = np.ascontiguousarray(
            xg.transpose(2, 0, 3, 1, 4).reshape(NG, 3 * GD, BLOC, L)
        )
        m = {"xrows": xp.reshape(NROWS, L), "xw3s": xw3s}
        m.update(tables)
        in_maps.append(m)
    return in_maps


def assemble_output(results):
    """results: list of 8 dicts with out2/out3/outs -> full [B, 2, 3, D, F]."""
    out = np.empty((B, 2, 3, D, F), np.float32)
    for c, res in enumerate(results):
        bs = slice(c * BLOC, (c + 1) * BLOC)
        # outs: [128, NT, 32] ; row r = t*128+p -> (b = r//DPAD, d = r%DPAD)
        sm = np.ascontiguousarray(res["outs"].transpose(1, 0, 2)).reshape(BLOC, DPAD, 32)[:, :D]
        out[bs, 0, 0] = sm[:, :, 0:8]     # ws1 max
        out[bs, 1, 0] = sm[:, :, 8:16]    # ws1 mean
        out[bs, 1, 1] = sm[:, :, 16:24]   # ws2 mean
        out[bs, 1, 2] = sm[:, :, 24:32]   # ws3 mean
        # out2/out3: [128, NG, BLOC]; partition m = di*F+f, free = b
        for nm, wsi in (("out2", 1), ("out3", 2)):
            o = res[nm].reshape(GD, F, NG, BLOC).transpose(3, 2, 0, 1).reshape(
                BLOC, DPAD, F
            )[:, :D]
            out[bs, 0, wsi] = o
    return out


def kernel(input_data, W1, b1, W2, b2, W3, b3):
    from concourse.bass_utils import run_bass_kernel_spmd

    nc = _get_program()
    in_maps = make_in_maps(input_data, W1, b1, W2, b2, W3, b3)
    res = run_bass_kernel_spmd(nc, in_maps, list(range(NCORES)))
    return assemble_output(res.results)
